# revision 2
# baseline (speedup 1.0000x reference)
"""Trainium2 Bass kernel for nn_AttentionConv2D (sparse_attention), v6.

The reference collapses (pos-never-incremented im2col bug) to

    out = (Wv x_s + bv) * w0,     w0 = e2 / (e2 + S8)      (zero at h=H-1 / w=W-1)
    e2  = exp(a0 - ln8),          a0 = x_q^T G x_s + u.x_s  (G = s Wq^T Wk)
    S8  = sum_j exp(U'_j . x_q + cp''_j)                    (j=1..8; v, c0, ln8 folded)

per pixel, where x_s is x shifted by (+1,+1).

Per core: one image, channels on partitions (2 chunks of 128), pixels
on the free dim, 4 pair-tiles of 1024 px.  The a0-path GEMM T' = G^T
x_q and the 8-logit projections run in fp8e4m3 with DoubleRow perf
mode (contract 256 channels per matmul, ~1.4x PE throughput); G is
pre-scaled by 256 and U' by 64 on the host (their entries sit below
the fp8 normal range), un-scaled for free via the ACT `scale` field.
The V GEMM and everything user-visible stays fp16.

PSUM (exactly 8 banks): tp[0], tp[1] (T' then V, [128,1024] f32, 2
banks each), a0/D ([128,1024], 2 banks), s1 ([16,1024], 2 banks).
A burst of tiny warm-up matmuls runs while the input DMAs land so the
PE's HAM clock-gate reaches 2.4 GHz before the real work starts; the
last pair is emitted softmax-chain-first / V-last to shorten the
dependency tail.
"""

import os
import sys

import numpy as np

for _p in ("/opt/trn_rl_repo",):
    if _p not in sys.path:
        sys.path.append(_p)

import concourse.bass as bass
import concourse.tile as tile
from concourse import bacc, mybir
from concourse import bass_utils

F32 = mybir.dt.float32
FP16 = mybir.dt.float16
FP8 = mybir.dt.float8e4
AF = mybir.ActivationFunctionType
ALU = mybir.AluOpType
DR = mybir.MatmulPerfMode.DoubleRow

B, C, H, W = 8, 256, 64, 64
HW = H * W              # 4096
A = 256
TW = 512                # pixels per matmul
PW = 1024               # pixels per pair-tile
NP = 4                  # pair-tiles per core
SHIFT = W + 1           # 65
BCOLS = PW + 68
XCOLS = HW + 68
WCOLS = A + 8 + A       # g | up | wvt packed per chunk (fp16 side)
W8C = 272               # g8 (256) | up8 (8) | pad (8), 16-aligned k-stride
SCALE = A ** -0.5
SG = 256.0              # fp8 pre-scale for G
SU = 64.0               # fp8 pre-scale for U'
LN8 = float(np.log(8.0))
NCORES = 8

_CACHE = {}
LAST_RESULTS = None


def _build():
    nc = bacc.Bacc("TRN2", target_bir_lowering=False, debug=False)

    x_d = nc.dram_tensor("x", [128, 2, XCOLS], FP16, kind="ExternalInput").ap()
    x8_d = nc.dram_tensor("x8", [128, 2, HW], FP8, kind="ExternalInput").ap()
    wts_d = nc.dram_tensor("wts", [128, 2 * WCOLS], FP16, kind="ExternalInput").ap()
    w8_d = nc.dram_tensor("w8", [128, 2, W8C], FP8, kind="ExternalInput").ap()
    uv_d = nc.dram_tensor("uv", [128, 4], F32, kind="ExternalInput").ap()
    cpp_d = nc.dram_tensor("cpp8", [8, 1], F32, kind="ExternalInput").ap()
    out_d = nc.dram_tensor("out", [128, 2, HW], FP16, kind="ExternalOutput").ap()

    with tile.TileContext(nc) as tc:
        with (
            tc.tile_pool(name="const", bufs=1) as const,
            tc.tile_pool(name="work", bufs=3) as work,
            tc.tile_pool(name="outp", bufs=2) as outp,
            tc.tile_pool(name="big", bufs=1, space="PSUM") as big,
            tc.tile_pool(name="psa0", bufs=1, space="PSUM") as psa0,
            tc.tile_pool(name="pss1", bufs=1, space="PSUM") as pss1,
        ):
            # ---- persistent inputs ----
            wts_sb = const.tile([128, 2 * WCOLS], FP16, name="wts", tag="wts")
            w8_sb = const.tile([128, 2, W8C], FP8, name="w8", tag="w8")
            x_sb = [const.tile([128, 2, BCOLS], FP16, name=f"xb{b}", tag=f"xb{b}")
                    for b in range(NP)]
            x8_sb = [const.tile([128, 2, PW], FP8, name=f"x8b{b}", tag=f"x8b{b}")
                     for b in range(NP)]
            uv_sb = const.tile([128, 4], F32, name="uv", tag="uv")
            cpp_sb = const.tile([8, 1], F32, name="cpp", tag="cpp")
            ones_sb = const.tile([128, 512], FP16, name="ones", tag="ones")
            nln8_sb = const.tile([128, 1], F32, name="nln8", tag="nln8")

            wvt = [wts_sb[:, k * WCOLS + A + 8:(k + 1) * WCOLS] for k in range(2)]
            u_ap = [uv_sb[:, k:k + 1] for k in range(2)]
            bv_ap = [uv_sb[:, 2 + k:3 + k] for k in range(2)]
            g8 = [w8_sb[:, :, a * 128:(a + 1) * 128] for a in range(2)]
            up8 = w8_sb[:, :, 256:272]

            nc.gpsimd.memset(ones_sb[:], 1.0)
            nc.gpsimd.memset(nln8_sb[:], -LN8)

            # scalar DMA queue: weights; sync queue: x blocks (b0 split so
            # the first matmuls' data lands first)
            nc.scalar.dma_start(w8_sb[:, :, :], w8_d[:, :, :])
            nc.scalar.dma_start(wts_sb[:], wts_d[:, :])
            nc.scalar.dma_start(uv_sb[:], uv_d[:, :])
            nc.scalar.dma_start(cpp_sb[:], cpp_d[:])
            nc.sync.dma_start(x8_sb[0][:, :, 0:TW], x8_d[:, :, 0:TW])
            nc.sync.dma_start(x8_sb[0][:, :, TW:PW], x8_d[:, :, TW:PW])
            nc.sync.dma_start(x_sb[0][:, :, 0:640], x_d[:, :, 0:640])
            nc.sync.dma_start(x_sb[0][:, :, 640:BCOLS], x_d[:, :, 640:BCOLS])
            for b in range(1, NP):
                nc.sync.dma_start(x8_sb[b][:, :, :], x8_d[:, :, b * PW:(b + 1) * PW])
                nc.sync.dma_start(x_sb[b][:, :, :], x_d[:, :, b * PW:b * PW + BCOLS])

            for P in range(NP):
                xs = [[x_sb[P][:, k, j * TW + SHIFT:j * TW + SHIFT + TW]
                       for j in range(2)] for k in range(2)]
                xsw = [x_sb[P][:, k, SHIFT:SHIFT + PW] for k in range(2)]
                x8t = [x8_sb[P][:, :, j * TW:(j + 1) * TW] for j in range(2)]

                tp = [big.tile([128, PW], F32, name=f"tp{a}", tag=f"tp{a}")
                      for a in range(2)]
                a0 = psa0.tile([128, PW], F32, name="a0", tag="a0")
                s1 = pss1.tile([16, PW], F32, name="s1", tag="s1")
                last = P == NP - 1

                # ---- T' = G^T x_q (fp8 DoubleRow, scaled by SG) ----
                for a in range(2):
                    for j in range(2):
                        nc.tensor.matmul(
                            tp[a][:, j * TW:(j + 1) * TW],
                            g8[a], x8t[j],
                            start=True, stop=True, perf_mode=DR,
                            skip_group_check=True,
                        )

                # ---- prod = (T' + SG*u) * x_s (evacuates T' banks) ----
                prod = [work.tile([128, PW], FP16, name=f"pr{a}", tag=f"pr{a}")
                        for a in range(2)]
                if P == 0:
                    # j-split so the chain starts on the first partial x DMA
                    for j in range(2):
                        hs = slice(j * TW, (j + 1) * TW)
                        for a in range(2):
                            nc.vector.scalar_tensor_tensor(
                                prod[a][:, hs], tp[a][:, hs], u_ap[a],
                                x_sb[P][:, a, j * TW + SHIFT:j * TW + SHIFT + TW],
                                ALU.add, ALU.mult,
                            )
                else:
                    for a in range(2):
                        nc.vector.scalar_tensor_tensor(
                            prod[a][:], tp[a][:], u_ap[a], xsw[a],
                            ALU.add, ALU.mult,
                        )

                # ---- s1 = U'^T x_q (fp8 DoubleRow, scaled by SU) ----
                for j in range(2):
                    nc.tensor.matmul(
                        s1[:, j * TW:(j + 1) * TW],
                        up8, x8t[j],
                        start=True, stop=True, perf_mode=DR,
                        skip_group_check=True,
                    )
                exp8 = work.tile([8, PW], FP16, name="e8", tag="e8")
                nc.scalar.activation(
                    exp8[:], s1[0:8, :], AF.Exp, bias=cpp_sb[:], scale=1.0 / SU,
                )

                # ---- V = Wv x_s chunk 0 (reuses tp[0]) ----
                if not last:
                    for k in range(2):
                        for j in range(2):
                            nc.tensor.matmul(
                                tp[0][:, j * TW:(j + 1) * TW],
                                wvt[k][:, 0:128], xs[k][j],
                                start=(k == 0), stop=(k == 1),
                                skip_group_check=True,
                            )

                # ---- a0 = colsum(prod) ----
                for a in range(2):
                    for j in range(2):
                        nc.tensor.matmul(
                            a0[:, j * TW:(j + 1) * TW],
                            ones_sb[:, 0:128], prod[a][:, j * TW:(j + 1) * TW],
                            start=(a == 0), stop=(a == 1),
                            skip_group_check=True,
                        )

                # ---- V chunk 1 (reuses tp[1]) ----
                if not last:
                    for k in range(2):
                        for j in range(2):
                            nc.tensor.matmul(
                                tp[1][:, j * TW:(j + 1) * TW],
                                wvt[k][:, 128:256], xs[k][j],
                                start=(k == 0), stop=(k == 1),
                                skip_group_check=True,
                            )

                # ---- e2 = exp(a0/SG - ln8) ----
                e2 = work.tile([128, PW], FP16, name="e2", tag="e2")
                nc.scalar.activation(
                    e2[:], a0[:], AF.Exp, bias=nln8_sb[:], scale=1.0 / SG,
                )

                # ---- D = S8 + e2 (reuses the a0 banks) ----
                for j in range(2):
                    nc.tensor.matmul(
                        a0[:, j * TW:(j + 1) * TW],
                        ones_sb[0:8, 0:128], exp8[:, j * TW:(j + 1) * TW],
                        start=True, stop=False, skip_group_check=True,
                    )
                for j in range(2):
                    nc.tensor.matmul(
                        a0[:, j * TW:(j + 1) * TW],
                        ones_sb[0:1, 0:128], e2[0:1, j * TW:(j + 1) * TW],
                        start=False, stop=True, skip_group_check=True,
                    )

                # ---- w0 = e2 / D ----
                rr = work.tile([128, PW], F32, name="rr", tag="rr")
                w0 = work.tile([128, PW], FP16, name="w0", tag="w0")
                nc.vector.reciprocal_approx_fast(rr[:], a0[:])
                if last or P == 0:
                    # keep the slow GPSIMD off the exposed dependency chains
                    # (pair 0 runs before the pipeline fills, the last pair
                    # is the closing tail)
                    nc.vector.tensor_tensor(w0[:], e2[:], rr[:], ALU.mult)
                else:
                    nc.gpsimd.tensor_mul(w0[:], e2[:], rr[:])

                # ---- last pair: V after the softmax chain (short tail) ----
                if last:
                    for aa in range(2):
                        for k in range(2):
                            for j in range(2):
                                nc.tensor.matmul(
                                    tp[aa][:, j * TW:(j + 1) * TW],
                                    wvt[k][:, aa * 128:(aa + 1) * 128], xs[k][j],
                                    start=(k == 0), stop=(k == 1),
                                    skip_group_check=True,
                                )

                # ---- vb = V + bv (both chunks on ACT) ----
                vb = [work.tile([128, PW], FP16, name=f"vb{a}", tag=f"vb{a}")
                      for a in range(2)]
                for a in range(2):
                    nc.scalar.activation(
                        vb[a][:], tp[a][:], AF.Identity, bias=bv_ap[a], scale=1.0,
                    )

                # ---- out = vb * w0: chunk 0 on DVE, chunk 1 on GPSIMD ----
                o2 = outp.tile([128, 2 * PW], FP16, name="o2", tag="o2")
                nc.vector.tensor_tensor(
                    o2[:, 0:PW], vb[0][:], w0[:], ALU.mult)
                if last:
                    nc.vector.tensor_tensor(
                        o2[:, PW:2 * PW], vb[1][:], w0[:], ALU.mult)
                else:
                    nc.gpsimd.tensor_mul(
                        o2[:, PW:2 * PW], vb[1][:], w0[:])
                nc.sync.dma_start(
                    out_d[:, :, P * PW:(P + 1) * PW], o2[:]
                )

    nc.compile()
    return nc


def _host_prep(x, Wq, bq, Wk, bk, Wv, bv):
    import ml_dtypes

    x = np.asarray(x, np.float32)
    Wq = np.asarray(Wq, np.float32)
    bq = np.asarray(bq, np.float32)
    Wk = np.asarray(Wk, np.float32)
    bk = np.asarray(bk, np.float32)
    Wv = np.asarray(Wv, np.float32)
    bv = np.asarray(bv, np.float32)

    pos = np.arange(9, dtype=np.float32)[:, None]
    div = np.exp(np.arange(0, C, 2, dtype=np.float32) * (-np.log(10000.0) / C))
    pe = np.zeros((9, C), np.float32)
    pe[:, 0::2] = np.sin(pos * div)
    pe[:, 1::2] = np.cos(pos * div)
    pe = pe.T

    bk2 = Wk @ pe[:, 0] + bk
    G = SCALE * (Wq.T @ Wk)
    v = SCALE * (Wq.T @ bk2)
    u = SCALE * (Wk.T @ bq)
    c0 = SCALE * float(bq @ bk2)
    kp = Wk @ pe[:, 1:] + bk[:, None]
    U = SCALE * (Wq.T @ kp)
    cp = SCALE * (bq @ kp)
    Up = U - v[:, None]
    cpp = cp - c0 - LN8

    f8 = ml_dtypes.float8_e4m3

    # fp16 packed weights [128, 2*WCOLS]: chunk k -> g | up | wvt
    # (g/up halves unused now but kept for layout simplicity)
    wts = np.concatenate([G, Up, Wv.T], axis=1).astype(np.float16)
    wts = np.ascontiguousarray(
        wts.reshape(2, 128, WCOLS).transpose(1, 0, 2).reshape(128, 2 * WCOLS))

    # fp8 packed stationary [128, 2, W8C]: G*SG | Up*SU | zero pad
    w8 = np.zeros((2, 128, W8C), np.float32)
    w8[:, :, :256] = (G * SG).reshape(2, 128, 256)
    w8[:, :, 256:264] = (Up * SU).reshape(2, 128, 8)
    w8 = np.ascontiguousarray(w8.transpose(1, 0, 2)).astype(f8)

    uv = np.zeros((128, 4), np.float32)
    uv[:, 0] = SG * u[:128]; uv[:, 1] = SG * u[128:]
    uv[:, 2] = bv[:128]; uv[:, 3] = bv[128:]

    xr = x.reshape(B, 2, 128, HW).transpose(0, 2, 1, 3)
    xp = np.zeros((B, 128, 2, XCOLS), np.float16)
    xp[:, :, :, :HW] = xr.astype(np.float16)
    x8 = np.ascontiguousarray(xr).astype(f8)

    common = {
        "wts": wts,
        "w8": w8,
        "uv": uv,
        "cpp8": np.ascontiguousarray(cpp[:, None].astype(np.float32)),
    }
    return [
        {"x": np.ascontiguousarray(xp[core]), "x8": x8[core], **common}
        for core in range(NCORES)
    ]


def kernel(x, Wq, bq, Wk, bk, Wv, bv):
    global LAST_RESULTS
    if "nc" not in _CACHE:
        _CACHE["nc"] = _build()
    nc = _CACHE["nc"]

    in_maps = _host_prep(x, Wq, bq, Wk, bk, Wv, bv)
    res = bass_utils.run_bass_kernel_spmd(
        nc, in_maps, core_ids=list(range(NCORES)),
        trace=bool(os.environ.get("KERNEL_TRACE")),
    )
    LAST_RESULTS = res
    out = np.stack([np.asarray(res.results[i]["out"]) for i in range(NCORES)], axis=0)
    out = out.astype(np.float32).transpose(0, 2, 1, 3).reshape(B, C, H, W)
    out[:, :, :, W - 1] = 0.0
    out[:, :, H - 1, :] = 0.0
    return out
